# revision 2
# baseline (speedup 1.0000x reference)
"""Trainium2 Bass kernel for nn_CausalUpsamplingLRU.

Causal upsampling LRU: 32 autoregressive passes of a diagonal complex LRU
over a 512-long sequence, feeding each pass's output back as the next input.

Strategy:
 - Data-parallel over batch: B=8 -> one batch element per NeuronCore.
 - Feature-major layout [features, seq] per core; everything SBUF-resident.
 - The diagonal complex recurrence h_t = lam*h_{t-1} + u_t is factored as
   lam = m * e^{i*theta}:  h_t = e^{i*theta*t} * s_t  with the REAL scan
   s_t = m*s_{t-1} + e^{-i*theta*t} u_t, done with the HW tensor_tensor_scan
   (fp32 state, per-partition multiplier m).
 - Phase rotations e^{+-i*theta*t} are elementwise multiplies with
   host-precomputed cos/sin tables, split across Vector and GpSimd engines.
 - The rotate-out adds are folded into the output matmul via PSUM
   accumulation (4 products per n-tile instead of rotate+2 products).
 - Matmuls run as float32r (TF32-class, 1 cycle/row) or float32 (4 cyc/row),
   selectable below.
"""
import sys
if '/opt/trn_rl_repo' not in sys.path:
    sys.path.insert(0, '/opt/trn_rl_repo')
import numpy as np
import concourse.bass as bass
import concourse.tile as tile
from concourse import bacc, mybir
from concourse.bass_utils import run_bass_kernel_spmd

F32 = mybir.dt.float32
F32R = mybir.dt.float32r
BF16 = mybir.dt.bfloat16
OP = mybir.AluOpType
ACT_COPY = mybir.ActivationFunctionType.Copy

B_SZ, SEQ, IN_CH, OUT_CH, STATE, OUT_SEQ = 8, 512, 256, 256, 384, 32
NT = STATE // 128   # 3 n-tiles
CT = IN_CH // 128   # 2 c-chunks
OT = OUT_CH // 128  # 2 o-tiles

USE_F32R = True     # matmul dtype: float32r (TF32-class) vs float32
N_REP = 1           # repeat whole program (timing experiments only)

_BUILD_CACHE = {}


def _build_nc():
    key = (USE_F32R, N_REP)
    if key in _BUILD_CACHE:
        return _BUILD_CACHE[key]
    MM_DT = F32R if USE_F32R else F32
    nc = bacc.Bacc("TRN2", target_bir_lowering=False, debug=False)

    # ---- DRAM I/O ----
    xT_d = nc.dram_tensor("xT", [IN_CH, SEQ], F32, kind="ExternalInput")
    bgre_d = nc.dram_tensor("BgReT", [IN_CH, STATE], F32, kind="ExternalInput")
    bgim_d = nc.dram_tensor("BgImT", [IN_CH, STATE], F32, kind="ExternalInput")
    cre_d = nc.dram_tensor("CReT", [STATE, OUT_CH], F32, kind="ExternalInput")
    cren_d = nc.dram_tensor("CReTn", [STATE, OUT_CH], F32, kind="ExternalInput")
    cimn_d = nc.dram_tensor("CImTn", [STATE, OUT_CH], F32, kind="ExternalInput")
    dt_d = nc.dram_tensor("DT", [IN_CH, OUT_CH], F32, kind="ExternalInput")
    cos_d = nc.dram_tensor("COS", [STATE, SEQ], F32, kind="ExternalInput")
    sin_d = nc.dram_tensor("SIN", [STATE, SEQ], F32, kind="ExternalInput")
    mcol_d = nc.dram_tensor("MCOL", [STATE, 1], F32, kind="ExternalInput")
    c512_d = nc.dram_tensor("C512", [STATE, 1], F32, kind="ExternalInput")
    s512_d = nc.dram_tensor("S512", [STATE, 1], F32, kind="ExternalInput")
    out_d = nc.dram_tensor("OUT", [OUT_CH, OUT_SEQ], F32, kind="ExternalOutput")

    with tile.TileContext(nc) as tc:
        with tc.tile_pool(name="const", bufs=1) as cp, \
             tc.tile_pool(name="xp", bufs=4) as xp, \
             tc.tile_pool(name="up", bufs=6, space="PSUM") as up, \
             tc.tile_pool(name="yp", bufs=2, space="PSUM") as yp, \
             tc.tile_pool(name="wp", bufs=6) as wp, \
             tc.tile_pool(name="sp", bufs=6) as sp, \
             tc.tile_pool(name="tp", bufs=6) as tp, \
             tc.tile_pool(name="ap", bufs=12) as apl, \
             tc.tile_pool(name="cyp", bufs=24) as cyp:

            # ---- persistent constants ----
            bgre = [cp.tile([128, STATE], MM_DT, tag=f"bgre{j}", name=f"bgre{j}") for j in range(CT)]
            bgim = [cp.tile([128, STATE], MM_DT, tag=f"bgim{j}", name=f"bgim{j}") for j in range(CT)]
            cre = [cp.tile([128, OUT_CH], MM_DT, tag=f"cre{j}", name=f"cre{j}") for j in range(NT)]
            cren = [cp.tile([128, OUT_CH], MM_DT, tag=f"cren{j}", name=f"cren{j}") for j in range(NT)]
            cimn = [cp.tile([128, OUT_CH], MM_DT, tag=f"cimn{j}", name=f"cimn{j}") for j in range(NT)]
            dtw = [cp.tile([128, OUT_CH], MM_DT, tag=f"dtw{j}", name=f"dtw{j}") for j in range(CT)]
            cost = [cp.tile([128, SEQ], F32, tag=f"cos{j}", name=f"cos{j}") for j in range(NT)]
            sint = [cp.tile([128, SEQ], F32, tag=f"sin{j}", name=f"sin{j}") for j in range(NT)]
            mcol = [cp.tile([128, 1], F32, tag=f"mcol{j}", name=f"mcol{j}") for j in range(NT)]
            c512 = [cp.tile([128, 1], F32, tag=f"c512{j}", name=f"c512{j}") for j in range(NT)]
            s512 = [cp.tile([128, 1], F32, tag=f"s512{j}", name=f"s512{j}") for j in range(NT)]
            zcol = cp.tile([128, 1], F32, tag="zcol")
            outb = [cp.tile([128, OUT_SEQ], F32, tag=f"outb{j}", name=f"outb{j}") for j in range(OT)]

            for j in range(CT):
                nc.sync.dma_start(out=bgre[j][:], in_=bgre_d[j*128:(j+1)*128, :].bitcast(MM_DT))
                nc.sync.dma_start(out=bgim[j][:], in_=bgim_d[j*128:(j+1)*128, :].bitcast(MM_DT))
                nc.sync.dma_start(out=dtw[j][:], in_=dt_d[j*128:(j+1)*128, :].bitcast(MM_DT))
            for j in range(NT):
                nc.sync.dma_start(out=cre[j][:], in_=cre_d[j*128:(j+1)*128, :].bitcast(MM_DT))
                nc.sync.dma_start(out=cren[j][:], in_=cren_d[j*128:(j+1)*128, :].bitcast(MM_DT))
                nc.sync.dma_start(out=cimn[j][:], in_=cimn_d[j*128:(j+1)*128, :].bitcast(MM_DT))
                nc.sync.dma_start(out=cost[j][:], in_=cos_d[j*128:(j+1)*128, :])
                nc.sync.dma_start(out=sint[j][:], in_=sin_d[j*128:(j+1)*128, :])
                nc.sync.dma_start(out=mcol[j][:], in_=mcol_d[j*128:(j+1)*128, :])
                nc.sync.dma_start(out=c512[j][:], in_=c512_d[j*128:(j+1)*128, :])
                nc.sync.dma_start(out=s512[j][:], in_=s512_d[j*128:(j+1)*128, :])
            nc.vector.memset(zcol[:], 0.0)

            for rep in range(N_REP):
                xa = [xp.tile([128, SEQ], MM_DT, tag="x", name="x") for _ in range(CT)]
                for j in range(CT):
                    nc.sync.dma_start(out=xa[j][:], in_=xT_d[j*128:(j+1)*128, :].bitcast(MM_DT))

                carry_re = [None]*NT
                carry_im = [None]*NT

                for it in range(OUT_SEQ):
                    # ---- U matmuls: u[n-tile] = Bg^T.T @ x ----
                    u_re, u_im = [], []
                    for jn in range(NT):
                        ur = up.tile([128, SEQ], F32, tag="u", name="u")
                        ui = up.tile([128, SEQ], F32, tag="u", name="u")
                        for jc in range(CT):
                            nc.tensor.matmul(ur[:], bgre[jc][:, jn*128:(jn+1)*128],
                                             xa[jc][:], start=(jc == 0), stop=(jc == CT-1))
                        for jc in range(CT):
                            nc.tensor.matmul(ui[:], bgim[jc][:, jn*128:(jn+1)*128],
                                             xa[jc][:], start=(jc == 0), stop=(jc == CT-1))
                        u_re.append(ur); u_im.append(ui)

                    # ---- y PSUM: D x accumulates first ----
                    yps = []
                    for jo in range(OT):
                        y = yp.tile([128, SEQ], F32, tag="y", name="y")
                        for jc in range(CT):
                            nc.tensor.matmul(y[:], dtw[jc][:, jo*128:(jo+1)*128],
                                             xa[jc][:], start=(jc == 0), stop=False)
                        yps.append(y)

                    s_re, s_im = [], []
                    for jn in range(NT):
                        # ---- rotate-in: w = e^{-i theta t} * u ----
                        t1 = tp.tile([128, SEQ], F32, tag="tmp", name="tmp")
                        t2 = tp.tile([128, SEQ], F32, tag="tmp", name="tmp")
                        w_re = wp.tile([128, SEQ], F32, tag="w", name="w")
                        nc.vector.tensor_tensor(t1[:], cost[jn][:], u_re[jn][:], OP.mult)
                        nc.vector.tensor_tensor(t2[:], sint[jn][:], u_im[jn][:], OP.mult)
                        nc.vector.tensor_tensor(w_re[:], t1[:], t2[:], OP.add)
                        t3 = tp.tile([128, SEQ], F32, tag="tmp", name="tmp")
                        t4 = tp.tile([128, SEQ], F32, tag="tmp", name="tmp")
                        w_im = wp.tile([128, SEQ], F32, tag="w", name="w")
                        nc.vector.tensor_tensor(t3[:], cost[jn][:], u_im[jn][:], OP.mult)
                        nc.vector.tensor_tensor(t4[:], sint[jn][:], u_re[jn][:], OP.mult)
                        nc.vector.tensor_tensor(w_im[:], t3[:], t4[:], OP.subtract)

                        # ---- real scans: s_t = m s_{t-1} + w_t ----
                        sr = sp.tile([128, SEQ], F32, tag="s", name="s")
                        si = sp.tile([128, SEQ], F32, tag="s", name="s")
                        d0 = mcol[jn][:].broadcast_to((128, SEQ))
                        init_r = 0.0 if it == 0 else carry_re[jn][:]
                        init_i = 0.0 if it == 0 else carry_im[jn][:]
                        nc.vector.tensor_tensor_scan(sr[:], d0, w_re[:], init_r,
                                                     OP.mult, OP.add)
                        nc.vector.tensor_tensor_scan(si[:], d0, w_im[:], init_i,
                                                     OP.mult, OP.add)
                        s_re.append(sr); s_im.append(si)

                        # ---- carry to next iteration: e^{i 512 theta} s_512 ----
                        if it < OUT_SEQ - 1:
                            q = cyp.tile([128, 1], F32, tag="cy", name="cy")
                            crn = cyp.tile([128, 1], F32, tag="cy", name="cy")
                            nc.vector.scalar_tensor_tensor(
                                q[:], si[:, SEQ-1:SEQ], s512[jn][:], zcol[:],
                                OP.mult, OP.add)
                            nc.vector.scalar_tensor_tensor(
                                crn[:], sr[:, SEQ-1:SEQ], c512[jn][:], q[:],
                                OP.mult, OP.subtract)
                            r2 = cyp.tile([128, 1], F32, tag="cy", name="cy")
                            cin = cyp.tile([128, 1], F32, tag="cy", name="cy")
                            nc.vector.scalar_tensor_tensor(
                                r2[:], sr[:, SEQ-1:SEQ], s512[jn][:], zcol[:],
                                OP.mult, OP.add)
                            nc.vector.scalar_tensor_tensor(
                                cin[:], si[:, SEQ-1:SEQ], c512[jn][:], r2[:],
                                OP.mult, OP.add)
                            carry_re[jn] = crn; carry_im[jn] = cin

                    # ---- rotated products (adds folded into C matmuls) ----
                    # y += Cre.(cos*s_re) - Cre.(sin*s_im) - Cim.(sin*s_re) - Cim.(cos*s_im)
                    for jn in range(NT):
                        a1 = apl.tile([128, SEQ], MM_DT, tag="a", name="a")
                        a2 = apl.tile([128, SEQ], MM_DT, tag="a", name="a")
                        a3 = apl.tile([128, SEQ], MM_DT, tag="a", name="a")
                        a4 = apl.tile([128, SEQ], MM_DT, tag="a", name="a")
                        nc.vector.tensor_tensor(a1[:], cost[jn][:], s_re[jn][:], OP.mult)
                        nc.vector.tensor_tensor(a2[:], sint[jn][:], s_im[jn][:], OP.mult)
                        nc.gpsimd.tensor_tensor(a3[:], sint[jn][:], s_re[jn][:], OP.mult)
                        nc.gpsimd.tensor_tensor(a4[:], cost[jn][:], s_im[jn][:], OP.mult)
                        for jo in range(OT):
                            last = (jn == NT-1)
                            o0 = jo*128
                            nc.tensor.matmul(yps[jo][:], cre[jn][:, o0:o0+128], a1[:],
                                             start=False, stop=False)
                            nc.tensor.matmul(yps[jo][:], cren[jn][:, o0:o0+128], a2[:],
                                             start=False, stop=False)
                            nc.tensor.matmul(yps[jo][:], cimn[jn][:, o0:o0+128], a3[:],
                                             start=False, stop=False)
                            nc.tensor.matmul(yps[jo][:], cimn[jn][:, o0:o0+128], a4[:],
                                             start=False, stop=last)

                    # ---- outputs + next x ----
                    if it < OUT_SEQ - 1:
                        xa = [xp.tile([128, SEQ], MM_DT, tag="x", name="x") for _ in range(CT)]
                        for jo in range(OT):
                            nc.scalar.activation(xa[jo][:], yps[jo][:], ACT_COPY)
                    for jo in range(OT):
                        nc.scalar.activation(outb[jo][:, it:it+1],
                                             yps[jo][:, SEQ-1:SEQ], ACT_COPY)

            for jo in range(OT):
                nc.sync.dma_start(out=out_d[jo*128:(jo+1)*128, :], in_=outb[jo][:])
    nc.compile()
    _BUILD_CACHE[key] = nc
    return nc


def _host_precompute(x, nu_log, theta_log, gamma_log, B_re, B_im, C_re, C_im, D):
    f8 = np.float64
    nu_log = np.asarray(nu_log, f8); theta_log = np.asarray(theta_log, f8)
    gamma_log = np.asarray(gamma_log, f8)
    B_re = np.asarray(B_re, f8); B_im = np.asarray(B_im, f8)
    C_re = np.asarray(C_re, f8); C_im = np.asarray(C_im, f8)
    D = np.asarray(D, f8)
    m = np.exp(-np.exp(nu_log)); theta = np.exp(theta_log)
    gamma = np.exp(gamma_log)
    t = np.arange(1, SEQ + 1, dtype=f8)[None, :]
    ang = theta[:, None] * t
    f4 = np.float32
    common = dict(
        BgReT=np.ascontiguousarray((gamma[:, None]*B_re).T.astype(f4)),
        BgImT=np.ascontiguousarray((gamma[:, None]*B_im).T.astype(f4)),
        CReT=np.ascontiguousarray(C_re.T.astype(f4)),
        CReTn=np.ascontiguousarray((-C_re.T).astype(f4)),
        CImTn=np.ascontiguousarray((-C_im.T).astype(f4)),
        DT=np.ascontiguousarray(D.T.astype(f4)),
        COS=np.cos(ang).astype(f4), SIN=np.sin(ang).astype(f4),
        MCOL=m.astype(f4)[:, None],
        C512=np.cos(theta*SEQ).astype(f4)[:, None],
        S512=np.sin(theta*SEQ).astype(f4)[:, None],
    )
    x = np.asarray(x, np.float32)
    in_maps = []
    for b in range(B_SZ):
        im = dict(common)
        im['xT'] = np.ascontiguousarray(x[b].T)
        in_maps.append(im)
    return in_maps


def kernel(x, nu_log, theta_log, gamma_log, B_re, B_im, C_re, C_im, D):
    nc = _build_nc()
    in_maps = _host_precompute(x, nu_log, theta_log, gamma_log,
                               B_re, B_im, C_re, C_im, D)
    res = run_bass_kernel_spmd(nc, in_maps, list(range(B_SZ)))
    out = np.stack([res.results[b]['OUT'].T for b in range(B_SZ)], axis=0)
    return np.ascontiguousarray(out.astype(np.float32))
